# revision 54
# baseline (speedup 1.0000x reference)
"""Adaptive-softmax NLL loss on 8 TRN2 NeuronCores.

Strategy: tensor-parallel over the vocab dimension. Head / tail1 GEMMs run
in fp8e4m3 DoubleRow mode (K=256 per pass, 0.5 cycles/row), tail2 in bf16.
Weights are pre-scaled by 32 on the host so fp8 stays in the normal range;
the exp activation compensates with scale=1/32. Each core holds its vocab
slice of W_head / W1 / W2 and all token activations resident in SBUF,
computes exp-sums of its logit slice (exp on the scalar engine into bf16
scratch, per-chunk sums on DVE), plus token-sharded gathered target-logit
dot products on DVE. Host permutes tokens so cluster-1 tokens occupy the
first T1 tiles and cluster-2 tokens the next T2 tiles -- tail work runs
only on those tiles. Two AllReduces (one hidden mid-kernel, one tiny at
the end) combine per-token sum-exp partials; every core then finishes the
scalar NLL identically.

NLL = sum_n log(S_head_n) + sum_{c1} log(S_t1_n) + sum_{c2} log(S_t2_n)
      - sum_n x_n . W_ext[cidx_n] - sum_{c1} h1_n . W1[t_n-C0]
      - sum_{c2} h2_n . W2[t_n-C1]

No max-subtraction needed: logits are O(1) by construction.
"""

import os
import sys

for _p in ("/opt/trn_rl_repo",):
    if _p not in sys.path:
        sys.path.insert(0, _p)

import numpy as np
import ml_dtypes

import concourse.bacc as bacc
import concourse.bass as bass
import concourse.bass_isa as bass_isa
import concourse.mybir as mybir
import concourse.tile as tile
from concourse.bass_utils import run_bass_kernel_spmd

dt = mybir.dt
AF = mybir.ActivationFunctionType
ALU = mybir.AluOpType
DR = mybir.MatmulPerfMode.DoubleRow

NCORES = 8
N, D = 4096, 1024
C0, C1, C2 = 20000, 40000, 50257
V1, V2 = C1 - C0, C2 - C1
VH = C0 + 2          # head logits incl 2 cluster columns
R1, R2 = 256, 64
VHC = 2560           # head vocab rows per core (8*2560 = 20480, pad 478)
V1C = 2560           # tail1 rows per core   (8*2560 = 20480, pad 480)
V2C = 1536           # tail2 rows per core   (8*1536 = 12288, pad 2031)
PAD_H = NCORES * VHC - VH
PAD_1 = NCORES * V1C - V1
PAD_2 = NCORES * V2C - V2
NT = N // 128        # 32 token tiles
NSH = N // NCORES    # 512 tokens per core for the sharded head dot
KD = D // 128        # 8 k-tiles over the D contraction
KK = KD // 2         # 4 DoubleRow passes over D
T1_DEF = 15          # tail1 token-tile capacity (1920 tokens, E[N1]=1630)
T2_DEF = 9           # tail2 token-tile capacity (1152 tokens, E[N2]=836)
NT_A = 24            # head tiles covered by the first (overlapped) AllReduce
WSC = 32.0           # fp8 weight pre-scale; exp() compensates by 1/WSC

F32, BF16, FP8 = dt.float32, dt.bfloat16, dt.float8e4
BF = ml_dtypes.bfloat16
F8 = ml_dtypes.float8_e4m3fn

# head / tail1 vocab chunking: (offset, width, psum tag)
CH_H = [(0, 1024, "A"), (1024, 1024, "A"), (2048, 512, "B")]
CH_2 = [(0, 1024, "A"), (1024, 512, "B")]
NCH_H = len(CH_H)    # slots per head / tail1 tile
NCH_2 = len(CH_2)    # slots per tail2 tile

LAST_EXEC_NS = None


def _build(T1, T2, OFF2):
    NTOK1, NTOK2 = T1 * 128, T2 * 128
    # first AllReduce fires at tile NTA-1 and must cover all tail slots
    NTA = NT_A if (T1 <= NT_A and OFF2 + T2 <= NT_A) else NT
    NTB = NT - NTA
    NOCC = bool(int(os.environ.get("KERNEL_NOCC", "0")))
    nc = bacc.Bacc("TRN2", target_bir_lowering=False, debug=False,
                   num_devices=NCORES)

    x_in = nc.declare_dram_parameter("x", [D, N], FP8, isOutput=False)
    whT = nc.declare_dram_parameter("whT", [D, VHC], FP8, isOutput=False)
    w1T = nc.declare_dram_parameter("w1T", [R1, V1C], FP8, isOutput=False)
    w2T = nc.declare_dram_parameter("w2T", [R2, V2C], BF16, isOutput=False)
    p1T = nc.declare_dram_parameter("p1T", [D, R1], FP8, isOutput=False)
    p2T = nc.declare_dram_parameter("p2T", [D, R2], FP8, isOutput=False)
    xTc = nc.declare_dram_parameter("xTc", [D, NSH], BF16, isOutput=False)
    wselT = nc.declare_dram_parameter("wselT", [D, NSH], BF16, isOutput=False)
    w1selT = nc.declare_dram_parameter("w1selT", [R1, NTOK1], BF16,
                                       isOutput=False)
    w2selT = nc.declare_dram_parameter("w2selT", [R2, NTOK2], BF16,
                                       isOutput=False)
    m1_in = nc.declare_dram_parameter("m1", [128, T1], F32, isOutput=False)
    m2_in = nc.declare_dram_parameter("m2", [128, T2], F32, isOutput=False)
    out_ext = nc.declare_dram_parameter("out", [1, 1], F32, isOutput=True)

    with tile.TileContext(nc) as tc:
        with (
            tc.tile_pool(name="res", bufs=1) as res,
            tc.tile_pool(name="dram", bufs=1, space="DRAM") as dram,
        ):
            # ---- resident loads (ordered: phase-1 needs first) --------------
            # x in fp8, 8 token-slices of 512 so phase 1 starts early
            x_sl = []
            for q in range(8):
                xs_ = res.tile([128, KD * 512], FP8, tag=f"x{q}",
                               name=f"x{q}")
                nc.sync.dma_start(
                    out=xs_[:].rearrange("p (k n) -> p k n", k=KD),
                    in_=x_in.ap().rearrange("(k p) n -> p k n", p=128)
                        [:, :, q * 512:(q + 1) * 512])
                x_sl.append(xs_)
                if q == 0:
                    p1_sb = res.tile([128, KD * R1], FP8)
                    nc.sync.dma_start(
                        out=p1_sb[:].rearrange("p (k r) -> p k r", k=KD),
                        in_=p1T.ap().rearrange("(k p) r -> p k r", p=128))
                    p2_sb = res.tile([128, KD * R2], FP8)
                    nc.sync.dma_start(
                        out=p2_sb[:].rearrange("p (k r) -> p k r", k=KD),
                        in_=p2T.ap().rearrange("(k p) r -> p k r", p=128))
                if q == 1:
                    # tail gathered weights: phase-1 PSUM dots need them early
                    w1sel_sb = res.tile([128, 2 * NTOK1], BF16)
                    nc.sync.dma_start(
                        out=w1sel_sb[:].rearrange("p (k n) -> p k n", k=2),
                        in_=w1selT.ap().rearrange("(k p) n -> p k n", p=128))
                    w2sel_sb = res.tile([64, NTOK2], BF16)
                    nc.sync.dma_start(out=w2sel_sb[:], in_=w2selT.ap())
                if q == 2:
                    # head gathered-dot inputs: consumed by gpsimd/scalar
                    # in their idle window well before the first exp
                    xTc_sb = res.tile([128, KD * NSH], BF16)
                    nc.sync.dma_start(
                        out=xTc_sb[:].rearrange("p (k n) -> p k n", k=KD),
                        in_=xTc.ap().rearrange("(k p) n -> p k n", p=128))
                    wsel_sb = res.tile([128, KD * NSH], BF16)
                    nc.sync.dma_start(
                        out=wsel_sb[:].rearrange("p (k n) -> p k n", k=KD),
                        in_=wselT.ap().rearrange("(k p) n -> p k n", p=128))
            def xdr(kk, n0, sz):
                """x DoubleRow AP [p, 2, sz] at DR pass kk, token n0."""
                q, off = divmod(n0, 512)
                return (x_sl[q][:].rearrange("p (k n) -> p k n", k=KD)
                        [:, 2 * kk:2 * kk + 2, off:off + sz])

            # big weights -- needed once phase 2 starts (~40us in)
            wh_sb = res.tile([128, KD * VHC], FP8)
            nc.sync.dma_start(
                out=wh_sb[:].rearrange("p (k v) -> p k v", k=KD),
                in_=whT.ap().rearrange("(k p) v -> p k v", p=128))
            wh4 = wh_sb[:].rearrange("p (k i v) -> p k i v", k=KK, i=2)
            w1_sb = res.tile([128, 2 * V1C], FP8)
            nc.sync.dma_start(
                out=w1_sb[:].rearrange("p (i v) -> p i v", i=2),
                in_=w1T.ap().rearrange("(i p) v -> p i v", p=128))
            w12 = w1_sb[:].rearrange("p (i v) -> p i v", i=2)

            w2_sb = res.tile([64, V2C], BF16)
            nc.sync.dma_start(out=w2_sb[:], in_=w2T.ap())
            m1_sb = res.tile([128, T1], F32)
            nc.sync.dma_start(out=m1_sb[:], in_=m1_in.ap())
            m2_sb = res.tile([128, T2], F32)
            nc.sync.dma_start(out=m2_sb[:], in_=m2_in.ap())

            h1f8 = res.tile([128, 2 * N], FP8)
            h18 = h1f8[:].rearrange("p (i n) -> p i n", i=2)
            h2T_sb = res.tile([64, N], BF16)

            # per-(tile, chunk) exp-sum slots
            shA = res.tile([128, NTA * NCH_H], F32)
            shB = res.tile([128, max(NTB, 1) * NCH_H], F32)
            s15 = res.tile([128, T1 * NCH_H], F32)
            s23 = res.tile([128, T2 * NCH_2], F32)
            dsh_slots = res.tile([128, KD], F32)   # sharded head dot partials
            dt1_slots = res.tile([128, 16], F32)   # tail1 dot partials (x32)
            dt2_slots = res.tile([64, 8], F32)     # tail2 dot partials (x32)
            nc.vector.memset(dt1_slots[:], 0.0)
            nc.vector.memset(dt2_slots[:], 0.0)

            # ---- phase 1: projections h1 = P1 @ x.T, h2 = P2 @ x.T (fp8).
            # Tail gathered-logit dots read the raw PSUM here (values are
            # 32*h, folded out of dgr/t2r at the end) ------------------------
            p1r = p1_sb[:].rearrange("p (k r) -> p k r", k=KD)
            p2r = p2_sb[:].rearrange("p (k r) -> p k r", k=KD)
            scrd = res.tile([128, 512], F32)
            scrd2 = res.tile([64, 512], F32)
            with tc.tile_pool(name="pj", bufs=2, space="PSUM") as pj:
                for q in range(8):           # 512-token chunks
                    n0 = q * 512
                    pa = pj.tile([128, 512], F32, tag="pa")
                    pb = pj.tile([128, 512], F32, tag="pb")
                    pc = pj.tile([64, 512], F32, tag="pc")
                    for kk in range(KK):
                        st = dict(start=(kk == 0), stop=(kk == KK - 1),
                                  perf_mode=DR)
                        rhs = xdr(kk, n0, 512)
                        nc.tensor.matmul(
                            pa[:], lhsT=p1r[:, 2 * kk:2 * kk + 2, 0:128],
                            rhs=rhs, **st)
                        nc.tensor.matmul(
                            pb[:], lhsT=p1r[:, 2 * kk:2 * kk + 2, 128:256],
                            rhs=rhs, **st)
                        nc.tensor.matmul(
                            pc[:], lhsT=p2r[:, 2 * kk:2 * kk + 2, 0:64],
                            rhs=rhs, **st)
                    qs = slice(n0, n0 + 512)
                    nc.vector.tensor_scalar_mul(h18[:, 0, qs], pa[:],
                                                1.0 / WSC)
                    nc.vector.tensor_scalar_mul(h18[:, 1, qs], pb[:],
                                                1.0 / WSC)
                    nc.vector.tensor_scalar_mul(h2T_sb[:, qs], pc[:],
                                                1.0 / WSC)
                    # tail1 dot partials over tokens [n0, n0+512) & [0,NTOK1)
                    w = min(NTOK1 - n0, 512)
                    if w > 0:
                        for k, pk in ((0, pa), (1, pb)):
                            nc.vector.tensor_mul(
                                scrd[:, 0:w], pk[:, 0:w],
                                w1sel_sb[:, k * NTOK1 + n0:
                                         k * NTOK1 + n0 + w])
                            nc.vector.reduce_sum(
                                dt1_slots[:, 2 * q + k:2 * q + k + 1],
                                scrd[:, 0:w], axis=mybir.AxisListType.X)
                    # tail2 dot partials over tokens cap [OFF2*128, +NTOK2)
                    lo = max(n0, OFF2 * 128)
                    hi = min(n0 + 512, OFF2 * 128 + NTOK2)
                    if lo < hi:
                        nc.vector.tensor_mul(
                            scrd2[:, 0:hi - lo], pc[:, lo - n0:hi - n0],
                            w2sel_sb[:, lo - OFF2 * 128:hi - OFF2 * 128])
                        nc.vector.reduce_sum(
                            dt2_slots[:, q:q + 1],
                            scrd2[:, 0:hi - lo], axis=mybir.AxisListType.X)

            # ---- head gathered-logit dot: gpsimd multiply + scalar Copy
            # accumulator -- both engines idle before the first exp (~45us) ---
            with tc.tile_pool(name="dsc", bufs=2) as dsc:
                for k in range(KD):
                    scr = dsc.tile([128, 512], F32, tag="s0")
                    nc.gpsimd.tensor_mul(
                        scr[:], xTc_sb[:, k * NSH:(k + 1) * NSH],
                        wsel_sb[:, k * NSH:(k + 1) * NSH])
                    nc.scalar.activation(scr[:], scr[:], AF.Copy,
                                         accum_out=dsh_slots[:, k:k + 1])

            # ---- phase 2: head + tail logits, exp, per-token sum-exp --------
            # dsh travels in the LAST AllReduce payload (B when present)
            WA = NTA + T1 + T2 + (0 if NTB else 1)
            pay_a = res.tile([128, WA], F32)
            red_a = res.tile([128, WA], F32)
            paydA = dram.tile([128, WA], F32)
            reddA = dram.tile([128, WA], F32)
            nllA = res.tile([128, 1], F32)
            logsA = res.tile([128, NTA + T1 + T2], F32)
            if NTB:
                pay_b = res.tile([128, NTB + 1], F32)
                red_b = res.tile([128, NTB + 1], F32)
                paydB = dram.tile([128, NTB + 1], F32)
                reddB = dram.tile([128, NTB + 1], F32)
                logsB = res.tile([128, NTB], F32)

            with tc.tile_pool(name="p2", bufs=4, space="PSUM") as p2p, \
                 tc.tile_pool(name="xsp", bufs=4) as xsp:
                for nt in range(NT):
                    n0 = nt * 128
                    sh = shA if nt < NTA else shB
                    sc = (nt if nt < NTA else nt - NTA) * NCH_H
                    for ci, (c0, cw, tg) in enumerate(CH_H):
                        pA = p2p.tile([128, cw], F32, tag=tg,
                                       bufs=(2 if cw == 1024 else 4))
                        for kk in range(KK):
                            for h in range(cw // 512):
                                nc.tensor.matmul(
                                    pA[:, h * 512:(h + 1) * 512],
                                    lhsT=xdr(kk, n0, 128),
                                    rhs=wh4[:, kk, :,
                                            c0 + h * 512:c0 + (h + 1) * 512],
                                    start=(kk == 0), stop=(kk == KK - 1),
                                    perf_mode=DR)
                        xs = xsp.tile([128, cw], BF16,
                                      tag=f"xs{cw}", bufs=2)
                        nc.scalar.activation(xs[:], pA[:], AF.Exp,
                                             scale=1.0 / WSC)
                        nc.vector.reduce_sum(sh[:, sc + ci:sc + ci + 1],
                                             xs[:], axis=mybir.AxisListType.X)
                    if nt < T1:
                        for ci, (c0, cw, tg) in enumerate(CH_H):
                            pA = p2p.tile([128, cw], F32, tag=tg,
                                       bufs=(2 if cw == 1024 else 4))
                            for h in range(cw // 512):
                                nc.tensor.matmul(
                                    pA[:, h * 512:(h + 1) * 512],
                                    lhsT=h18[:, :, n0:n0 + 128],
                                    rhs=w12[:, :,
                                            c0 + h * 512:c0 + (h + 1) * 512],
                                    start=True, stop=True, perf_mode=DR)
                            xs = xsp.tile([128, cw], BF16,
                                          tag=f"xs{cw}", bufs=2)
                            nc.scalar.activation(xs[:], pA[:], AF.Exp,
                                                 scale=1.0 / WSC)
                            nc.vector.reduce_sum(
                                s15[:, nt * NCH_H + ci:nt * NCH_H + ci + 1],
                                xs[:], axis=mybir.AxisListType.X)
                    if OFF2 <= nt < OFF2 + T2:
                        for ci, (c0, cw, tg) in enumerate(CH_2):
                            pA = p2p.tile([128, cw], F32, tag=tg,
                                       bufs=(2 if cw == 1024 else 4))
                            for h in range(cw // 512):
                                nc.tensor.matmul(
                                    pA[:, h * 512:(h + 1) * 512],
                                    lhsT=h2T_sb[:, n0:n0 + 128],
                                    rhs=w2_sb[:,
                                              c0 + h * 512:c0 + (h + 1) * 512],
                                    start=True, stop=True)
                            xs = xsp.tile([128, cw], BF16,
                                          tag=f"xs{cw}", bufs=2)
                            nc.scalar.activation(xs[:], pA[:], AF.Exp)
                            nc.vector.reduce_sum(
                                s23[:, (nt - OFF2) * NCH_2 + ci:
                                    (nt - OFF2) * NCH_2 + ci + 1],
                                xs[:], axis=mybir.AxisListType.X)

                    if nt == NTA - 1:
                        # ---- first AllReduce: head tiles 0..NTA-1 + tails ---
                        shA5 = shA[:].rearrange("p (t v) -> p t v", v=NCH_H)
                        nc.vector.tensor_add(pay_a[:, 0:NTA], shA5[:, :, 0],
                                             shA5[:, :, 1])
                        for v in range(2, NCH_H):
                            nc.vector.tensor_add(pay_a[:, 0:NTA],
                                                 pay_a[:, 0:NTA],
                                                 shA5[:, :, v])
                        s155 = s15[:].rearrange("p (t v) -> p t v", v=NCH_H)
                        nc.vector.tensor_add(pay_a[:, NTA:NTA + T1],
                                             s155[:, :, 0], s155[:, :, 1])
                        for v in range(2, NCH_H):
                            nc.vector.tensor_add(pay_a[:, NTA:NTA + T1],
                                                 pay_a[:, NTA:NTA + T1],
                                                 s155[:, :, v])
                        s233 = s23[:].rearrange("p (t v) -> p t v", v=NCH_2)
                        nc.vector.tensor_add(pay_a[:, NTA + T1:NTA + T1 + T2],
                                             s233[:, :, 0], s233[:, :, 1])
                        for v in range(2, NCH_2):
                            nc.vector.tensor_add(
                                pay_a[:, NTA + T1:NTA + T1 + T2],
                                pay_a[:, NTA + T1:NTA + T1 + T2],
                                s233[:, :, v])
                        if not NTB:
                            nc.vector.reduce_sum(
                                pay_a[:, NTA + T1 + T2:NTA + T1 + T2 + 1],
                                dsh_slots[:], axis=mybir.AxisListType.X)
                        nc.sync.dma_start(out=paydA[:], in_=pay_a[:])
                        if NOCC:
                            nc.sync.dma_start(out=reddA[:], in_=paydA[:])
                        else:
                            nc.gpsimd.collective_compute(
                                "AllReduce", ALU.add,
                                replica_groups=[list(range(NCORES))],
                                ins=[paydA.opt()], outs=[reddA.opt()])
                        nc.sync.dma_start(out=red_a[:], in_=reddA[:])

            # ---- post-loop: log/mask math for AR-A (overlapped AR done) -----
            nc.vector.tensor_scalar_add(
                logsA[:, 0:NTA], red_a[:, 0:NTA], float(-PAD_H))
            nc.vector.tensor_scalar_add(
                logsA[:, NTA:NTA + T1],
                red_a[:, NTA:NTA + T1], float(-PAD_1))
            nc.vector.tensor_scalar_add(
                logsA[:, NTA + T1:NTA + T1 + T2],
                red_a[:, NTA + T1:NTA + T1 + T2], float(-PAD_2))
            nc.scalar.activation(logsA[:], logsA[:], AF.Ln)
            nc.vector.tensor_mul(logsA[:, NTA:NTA + T1],
                                 logsA[:, NTA:NTA + T1], m1_sb[:])
            nc.vector.tensor_mul(
                logsA[:, NTA + T1:NTA + T1 + T2],
                logsA[:, NTA + T1:NTA + T1 + T2], m2_sb[:])
            nc.vector.reduce_sum(nllA[:], logsA[:],
                                 axis=mybir.AxisListType.X)
            # everything not depending on AR-B, folded in before it lands
            tot = res.tile([128, 1], F32)
            dgr = res.tile([128, 1], F32)
            nc.vector.reduce_sum(dgr[:], dt1_slots[:],
                                 axis=mybir.AxisListType.X)
            nc.vector.tensor_scalar_mul(dgr[:], dgr[:], 1.0 / WSC)
            nc.vector.tensor_sub(tot[:], nllA[:], dgr[:])
            t2r = res.tile([64, 1], F32)
            nc.vector.reduce_sum(t2r[:], dt2_slots[:],
                                 axis=mybir.AxisListType.X)
            nc.vector.tensor_scalar_mul(t2r[:], t2r[:], 1.0 / WSC)
            nc.vector.tensor_sub(tot[:64, :], tot[:64, :], t2r[:])
            if not NTB:
                nc.vector.tensor_sub(
                    tot[:], tot[:],
                    red_a[:, NTA + T1 + T2:NTA + T1 + T2 + 1])

            # ---- final: second AllReduce (remaining head tiles + dsh) -------
            if NTB:
                shB5 = shB[:].rearrange("p (t v) -> p t v", v=NCH_H)
                nc.vector.tensor_add(pay_b[:, 0:NTB], shB5[:, :, 0],
                                     shB5[:, :, 1])
                for v in range(2, NCH_H):
                    nc.vector.tensor_add(pay_b[:, 0:NTB], pay_b[:, 0:NTB],
                                         shB5[:, :, v])
                nc.vector.reduce_sum(pay_b[:, NTB:NTB + 1], dsh_slots[:],
                                     axis=mybir.AxisListType.X)
                nc.sync.dma_start(out=paydB[:], in_=pay_b[:])
                if NOCC:
                    nc.sync.dma_start(out=reddB[:], in_=paydB[:])
                else:
                    nc.gpsimd.collective_compute(
                        "AllReduce", ALU.add,
                        replica_groups=[list(range(NCORES))],
                        ins=[paydB.opt()], outs=[reddB.opt()])
                nc.sync.dma_start(out=red_b[:], in_=reddB[:])
                nc.vector.tensor_scalar_add(logsB[:], red_b[:, 0:NTB],
                                            float(-PAD_H))
                nc.scalar.activation(logsB[:], logsB[:], AF.Ln)
                nllB = res.tile([128, 1], F32)
                nc.vector.reduce_sum(nllB[:], logsB[:],
                                     axis=mybir.AxisListType.X)
                nc.vector.tensor_add(tot[:], tot[:], nllB[:])
                nc.vector.tensor_sub(tot[:], tot[:],
                                     red_b[:, NTB:NTB + 1])
            par = res.tile([128, 1], F32)
            nc.gpsimd.partition_all_reduce(par[:], tot[:], channels=128,
                                           reduce_op=bass_isa.ReduceOp.add)
            nc.sync.dma_start(out=out_ext.ap(), in_=par[0:1, :])

    nc.compile()
    return nc


_NC = {}


def _get_nc(T1, T2, OFF2):
    key = (T1, T2, OFF2)
    if key not in _NC:
        _NC[key] = _build(T1, T2, OFF2)
    return _NC[key]


def _prepare(inputs):
    x = np.ascontiguousarray(inputs["x"], dtype=np.float32)
    target = np.asarray(inputs["target"]).astype(np.int64)
    W_head = np.asarray(inputs["W_head"], dtype=np.float32)
    W_cluster = np.asarray(inputs["W_cluster"], dtype=np.float32)
    P1 = np.asarray(inputs["P1"], dtype=np.float32)
    W1 = np.asarray(inputs["W1"], dtype=np.float32)
    P2 = np.asarray(inputs["P2"], dtype=np.float32)
    W2 = np.asarray(inputs["W2"], dtype=np.float32)

    # ---- host-side sharding / permutation / index gathers ------------------
    mask1 = (target >= C0) & (target < C1)
    mask2 = target >= C1
    mask0 = ~(mask1 | mask2)
    idx1 = np.nonzero(mask1)[0]
    idx2 = np.nonzero(mask2)[0]
    idx0 = np.nonzero(mask0)[0]
    N1, N2 = len(idx1), len(idx2)
    T1 = max(T1_DEF, -(-N1 // 128))
    T2 = max(T2_DEF, -(-N2 // 128))
    if T1 * 128 + T2 * 128 <= N:
        OFF2 = T1
        NTOK1, NTOK2 = T1 * 128, T2 * 128
        # layout: [cluster1 | pad0 | cluster2 | rest of cluster0]
        perm = np.empty(N, dtype=np.int64)
        g1 = NTOK1 - N1                   # cluster-0 fill between 1 and 2
        perm[0:N1] = idx1
        perm[N1:NTOK1] = idx0[:g1]
        perm[NTOK1:NTOK1 + N2] = idx2
        perm[NTOK1 + N2:] = idx0[g1:]
    else:
        # degenerate fallback: no permutation, tails run over all tiles
        T1 = T2 = NT
        OFF2 = 0
        NTOK1, NTOK2 = N, N
        perm = np.arange(N, dtype=np.int64)

    xp = x[perm]
    tp = target[perm]
    m1p = (tp[:NTOK1] >= C0) & (tp[:NTOK1] < C1)
    m2p = tp[OFF2 * 128:OFF2 * 128 + NTOK2] >= C1
    cidx = np.where(tp < C0, tp,
                    np.where(tp < C1, C0, C0 + 1)).astype(np.int64)
    W_ext = np.concatenate([W_head, W_cluster], axis=0)          # [20002, D]
    W1p = np.concatenate([W1, np.zeros((1, R1), np.float32)], axis=0)
    W2p = np.concatenate([W2, np.zeros((1, R2), np.float32)], axis=0)
    j1 = np.where(m1p, tp[:NTOK1] - C0, V1).astype(np.int64)
    j2 = np.where(m2p, tp[OFF2 * 128:OFF2 * 128 + NTOK2] - C1,
                  V2).astype(np.int64)

    xT = np.ascontiguousarray(xp.T)                              # [D, N] f32
    xT8 = np.ascontiguousarray(xT.astype(F8))
    xTb = np.ascontiguousarray(xT.astype(BF))
    WhT_full = np.zeros((NCORES * VHC, D), np.float32)
    WhT_full[:VH] = W_ext
    W1_full = np.zeros((NCORES * V1C, R1), np.float32)
    W1_full[:V1] = W1
    W2_full = np.zeros((NCORES * V2C, R2), np.float32)
    W2_full[:V2] = W2
    wselT = np.ascontiguousarray(W_ext[cidx].T.astype(BF))       # [D, N]
    w1selT = np.ascontiguousarray(W1p[j1].T.astype(BF))          # [R1, NTOK1]
    w2selT = np.ascontiguousarray(W2p[j2].T.astype(BF))          # [R2, NTOK2]
    m1 = np.ascontiguousarray(
        m1p.astype(np.float32).reshape(T1, 128).T)               # [128, T1]
    m2 = np.ascontiguousarray(
        m2p.astype(np.float32).reshape(T2, 128).T)

    in_maps = []
    for i in range(NCORES):
        in_maps.append({
            "x": xT8,
            "whT": np.ascontiguousarray(
                (WhT_full[i * VHC:(i + 1) * VHC].T * WSC).astype(F8)),
            "w1T": np.ascontiguousarray(
                (W1_full[i * V1C:(i + 1) * V1C].T * WSC).astype(F8)),
            "w2T": np.ascontiguousarray(
                W2_full[i * V2C:(i + 1) * V2C].T.astype(BF)),
            "p1T": np.ascontiguousarray((P1.T * WSC).astype(F8)),
            "p2T": np.ascontiguousarray((P2.T * WSC).astype(F8)),
            "xTc": np.ascontiguousarray(xTb[:, i * NSH:(i + 1) * NSH]),
            "wselT": np.ascontiguousarray(wselT[:, i * NSH:(i + 1) * NSH]),
            "w1selT": w1selT,
            "w2selT": w2selT,
            "m1": m1,
            "m2": m2,
        })
    return in_maps, T1, T2, OFF2


def kernel(**inputs):
    in_maps, T1, T2, OFF2 = _prepare(inputs)
    nc = _get_nc(T1, T2, OFF2)
    trace = bool(int(os.environ.get("KERNEL_TRACE", "0")))
    if trace:
        _install_ntff_hook()
    res = run_bass_kernel_spmd(nc, in_maps, core_ids=list(range(NCORES)),
                               trace=trace)
    global LAST_EXEC_NS
    LAST_EXEC_NS = res.exec_time_ns
    val = np.float32(res.results[0]["out"][0, 0])
    return np.asarray(val, dtype=np.float32)


def _install_ntff_hook():
    """Shim antenv.axon_hooks so trace=True can capture NTFF profiles."""
    import types
    import antenv
    if hasattr(antenv, "axon_hooks"):
        return
    hooks = types.ModuleType("antenv.axon_hooks")
    holder = [None]
    hooks.set_axon_ntff_profile_hook = lambda h: holder.__setitem__(0, h)
    hooks.get_axon_ntff_profile_hook = lambda: holder[0]
    sys.modules["antenv.axon_hooks"] = hooks
    antenv.axon_hooks = hooks
    try:
        from trn_agent_boot.trn_boot import _ntff_profile_via_ctypes
        hooks.set_axon_ntff_profile_hook(
            _ntff_profile_via_ctypes("/opt/axon/libaxon_pjrt.so"))
    except Exception:
        pass
